# revision 40
# baseline (speedup 1.0000x reference)
"""Trainium2 Bass kernel for BaseAttnPredictNet (pre-LN MHA with zero-attn
slot, gated output combination, residual).

v7 strategy (on top of host mask-compaction + fp8 PV + natural epilogue):
- k/v/qT are loaded PRE-TRANSPOSED from HBM via DMA-transpose; no natural
  staging, no PE transposes, no DVE layernorm passes for k/v.
- k/v LN is folded: row mean/meansq come from 1/512-valued ones matmuls on
  the PE, rstd = exp(-0.5*ln(var+eps)) on ACT, the mean correction enters
  the k/v projections as a rank-1 accumulation (-colsum(W) (x) mean), and
  the rstd scale rides the exp's per-partition scale (keys are partitions
  in the score block) resp. the vh copy scale.
- Attention: QK head-pair row-tiled bf16 matmuls -> shifted exp to fp8 ->
  DoubleRow fp8 PV with a valid-column producing the softmax denominator.
- Pair nb+1's k/q projections and the whole v projection pipeline are
  interleaved into the exp-paced attention stream.
- Epilogue in natural layout via activation-as-lhsT matmuls; gate bias via
  ones-row rank-1; all-bf16 combine.
"""

import numpy as np
import ml_dtypes

import concourse.bass as bass
import concourse.bacc as bacc
import concourse.mybir as mybir
import concourse.tile as tile
from concourse.bass_utils import run_bass_kernel_spmd
from concourse.masks import make_identity

B, Q, KLEN, D = 2, 2048, 2048, 512
H, DH = 8, 64
P = 128
QS = 512
ND = D // P
NCORES = 8
SCALE = 0.125
LN_EPS = 1e-5
ESHIFT = -4.5  # keeps exp within fp8 e4m3 range; softmax-invariant

F32 = mybir.dt.float32
BF16 = mybir.dt.bfloat16
AF = mybir.ActivationFunctionType
OP = mybir.AluOpType
BF = ml_dtypes.bfloat16
FP8 = mybir.dt.float8e4


def _build(QA: int, KC: int) -> bass.Bass:
    NKC = KC // P
    nc = bacc.Bacc("TRN2", target_bir_lowering=False, debug=False)

    din = {}
    for name, shape, dt in (
        ("qf", [QS, D], BF16),
        ("kc", [KC, D], BF16),
        ("vc", [KC, D], BF16),
        ("wq", [D, D], BF16),
        ("wk", [D, D], BF16),
        ("wv", [D, D], BF16),
        ("wo", [D, D], BF16),
        ("wop", [D, D], BF16),
        ("gwq", [D, D], BF16),
        ("gbn", [1, D], BF16),
        ("w1kn", [1, D], BF16),
        ("w1vn", [1, D], BF16),
        ("kval", [P, NKC], F32),
        ("qm", [1, QS], F32),
    ):
        din[name] = nc.dram_tensor(name, shape, dt, kind="ExternalInput")
    out_d = nc.dram_tensor("out", [QS, D], BF16, kind="ExternalOutput")

    with tile.TileContext(nc) as tc:
        _body(nc, tc, din, out_d, QA, KC)
    nc.compile()
    return nc


def _body(nc, tc, din, out_d, QA, KC):
    NKC = KC // P
    JC = KC // 3
    from contextlib import ExitStack

    ctx = ExitStack()
    with ctx:
        persist = ctx.enter_context(tc.tile_pool(name="persist", bufs=1))
        stats = ctx.enter_context(tc.tile_pool(name="stats", bufs=6))

        ident_bf = persist.tile([P, P], BF16)
        make_identity(nc, ident_bf)
        ident_f32 = persist.tile([P, P], F32)
        make_identity(nc, ident_f32)
        ones_bf = persist.tile([P, P], BF16)
        nc.vector.memset(ones_bf, 1.0)
        oneN = persist.tile([P, 1], BF16)
        nc.vector.memset(oneN, 1.0 / D)
        eps_t = persist.tile([P, 1], F32)
        nc.vector.memset(eps_t, LN_EPS)
        eshift_t = persist.tile([P, 1], F32)
        nc.vector.memset(eshift_t, ESHIFT)

        kval = persist.tile([P, NKC], F32)
        gbn = persist.tile([1, D], BF16)
        w1kn = persist.tile([1, D], BF16)
        w1vn = persist.tile([1, D], BF16)
        qm_bc = persist.tile([P, QS], F32)

        # ---- persistent activations ----
        kT = persist.tile([P, ND, KC], BF16)   # raw k, feature-major
        vT = persist.tile([P, ND, KC], BF16)
        vh_aug = persist.tile([P, NKC, H, DH + 2], FP8)
        qnT = persist.tile([P, ND, QA], BF16)
        qT = persist.tile([P, ND, QS], BF16)
        qhT = [persist.tile([P, QA], BF16, name=f"qhT{a}") for a in range(ND)]
        khT = [persist.tile([P, KC], BF16, name=f"khT{a}") for a in range(ND)]
        avT = persist.tile([P, ND, QA], BF16)
        po = persist.tile([P, ND, D], BF16)
        g_nat = persist.tile([P, ND, D], BF16)
        outn = persist.tile([P, ND, D], BF16)
        qf_s = persist.tile([P, ND, D], BF16)
        esck = persist.tile([P, NKC], F32)   # exp scale: SCALE*rstd_k
        vscv = persist.tile([P, NKC], F32)   # vh scale: rstd_v*kval

        # ---- DMAs: transposed k/v/qT, natural qf, weights ----
        for b in range(ND):
            nc.sync.dma_start_transpose(
                out=kT[:, b, :], in_=din["kc"][:, b * P : (b + 1) * P]
            )
        nc.sync.dma_start(
            out=qf_s, in_=din["qf"][:, :].rearrange("(a p) d -> p a d", p=P)
        )
        for b in range(ND):
            nc.sync.dma_start_transpose(
                out=qT[:, b, :], in_=din["qf"][:, b * P : (b + 1) * P]
            )
        for b in range(ND):
            nc.sync.dma_start_transpose(
                out=vT[:, b, :], in_=din["vc"][:, b * P : (b + 1) * P]
            )
        nc.sync.dma_start(out=kval, in_=din["kval"][:, :])
        nc.sync.dma_start(out=gbn, in_=din["gbn"][:, :])
        nc.sync.dma_start(out=w1kn, in_=din["w1kn"][:, :])
        nc.sync.dma_start(out=w1vn, in_=din["w1vn"][:, :])
        _qm_ap = din["qm"][:, :]
        nc.sync.dma_start(
            out=qm_bc,
            in_=bass.AP(tensor=_qm_ap.tensor, offset=_qm_ap.offset, ap=[[0, P], [1, QS]]),
        )
        w_s = {}
        for wname in ("wk", "wq", "wv", "wo", "wop", "gwq"):
            wt = persist.tile([P, ND, D], BF16, name=f"{wname}_s")
            nc.sync.dma_start(
                out=wt, in_=din[wname][:, :].rearrange("(b p) d -> p b d", p=P)
            )
            w_s[wname] = wt

        ptp = tc.alloc_tile_pool(name="ptp", bufs=1, space="PSUM")
        pp = tc.alloc_tile_pool(name="pp", bufs=2, space="PSUM")
        pst = tc.alloc_tile_pool(name="pst", bufs=2, space="PSUM")

        def fold_stats(xT, dst_pn):
            """Row LN stats from feature-major xT: means (returned as [1,KC]
            SBUF bf16 for rank-1 use) and rstd transposed into dst_pn [P,NKC]
            via a DRAM round-trip."""
            m_row = stats.tile([1, KC], F32, name="m_row", bufs=2)
            ms_row = stats.tile([1, KC], F32, name="ms_row", bufs=2)
            sq = stats.tile([P, ND, JC], BF16, name="sq", bufs=2)
            for j0 in range(0, KC, JC):
                psm = pst.tile([1, 512], F32, name="pst_t")
                for b in range(ND):
                    nc.tensor.matmul(
                        psm[:, :JC],
                        oneN[:, :],
                        xT[:, b, j0 : j0 + JC],
                        start=(b == 0),
                        stop=(b == ND - 1),
                    )
                nc.vector.tensor_copy(m_row[:, j0 : j0 + JC], psm[:, :JC])
                for b in range(ND):
                    nc.scalar.activation(
                        out=sq[:, b, :], in_=xT[:, b, j0 : j0 + JC], func=AF.Square
                    )
                psq = pst.tile([1, 512], F32, name="pst_t")
                for b in range(ND):
                    nc.tensor.matmul(
                        psq[:, :JC],
                        oneN[:, :],
                        sq[:, b, :],
                        start=(b == 0),
                        stop=(b == ND - 1),
                    )
                nc.vector.tensor_copy(ms_row[:, j0 : j0 + JC], psq[:, :JC])
            m_bf = stats.tile([1, KC], BF16, name="m_bf", bufs=2)
            nc.vector.tensor_copy(m_bf, m_row)
            var = stats.tile([1, KC], F32, name="var", bufs=2)
            nc.vector.tensor_tensor(out=var, in0=m_row, in1=m_row, op=OP.mult)
            nc.vector.tensor_tensor(out=var, in0=ms_row, in1=var, op=OP.subtract)
            rst_row = stats.tile([1, KC], F32, name="rst_row", bufs=2)
            nc.scalar.activation(
                out=rst_row, in_=var, func=AF.Abs_reciprocal_sqrt, bias=eps_t[0:1, :]
            )
            # layout flip [1,KC] -> [P,NKC]: tiny PE transposes, one per block
            prt = pst.tile([P, NKC], F32, name="pst_f")
            for c in range(NKC):
                nc.tensor.transpose(
                    prt[:, c : c + 1],
                    rst_row[0:1, c * P : (c + 1) * P],
                    ident_f32[0:1, 0:1],
                )
            nc.vector.tensor_copy(dst_pn, prt)
            return m_bf

        # ---- k: stats + projection (a=0 now; a>0 interleaved later) ----
        km_bf = fold_stats(kT, esck)
        nc.vector.tensor_scalar_mul(esck, esck, SCALE)

        def kproj_chunk(a, j0, pool, copy_eng):
            ps = pool.tile([P, 512], F32, name="prj_t")
            for b in range(ND):
                nc.tensor.matmul(
                    ps[:, :JC],
                    w_s["wk"][:, b, a * P : (a + 1) * P],
                    kT[:, b, j0 : j0 + JC],
                    start=(b == 0),
                    stop=False,
                )
            nc.tensor.matmul(
                ps[:, :JC],
                w1kn[0:1, a * P : (a + 1) * P],
                km_bf[0:1, j0 : j0 + JC],
                start=False,
                stop=True,
            )
            if copy_eng == "act":
                nc.scalar.copy(khT[a][:, j0 : j0 + JC], ps[:, :JC])
            else:
                nc.vector.tensor_copy(khT[a][:, j0 : j0 + JC], ps[:, :JC])

        def qproj_a(a, pool, copy_eng):
            ps = pool.tile([P, 512], F32, name="prj_t")
            for b in range(ND):
                nc.tensor.matmul(
                    ps[:, :QA],
                    w_s["wq"][:, b, a * P : (a + 1) * P],
                    qnT[:, b, :],
                    start=(b == 0),
                    stop=(b == ND - 1),
                )
            if copy_eng == "act":
                nc.scalar.copy(qhT[a], ps[:, :QA])
            else:
                nc.vector.tensor_copy(qhT[a], ps[:, :QA])

        for j0 in range(0, KC, JC):
            kproj_chunk(0, j0, pp, "act")

        # ---- v stats (vproj itself is interleaved into attention nb=0) ----
        vm_bf = fold_stats(vT, vscv)
        nc.vector.tensor_tensor(out=vscv, in0=vscv, in1=kval, op=OP.mult)

        def vproj_block(c, pool):
            ps = pool.tile([P, 512], F32, name="prj_t")
            for b in range(ND):
                nc.tensor.matmul(
                    ps,
                    vT[:, b, c * P : (c + 1) * P],
                    w_s["wv"][:, b, :],
                    start=(b == 0),
                    stop=False,
                )
            nc.tensor.matmul(
                ps,
                vm_bf[0:1, c * P : (c + 1) * P],
                w1vn[0:1, :],
                start=False,
                stop=True,
            )
            pp3 = ps.rearrange("p (h e) -> p h e", h=H)
            nc.vector.tensor_scalar(
                out=vh_aug[:, c, :, 0:DH],
                in0=pp3,
                scalar1=vscv[:, c : c + 1],
                scalar2=None,
                op0=OP.mult,
            )
            nc.vector.tensor_copy(
                vh_aug[:, c, :, DH : DH + 1],
                kval[:, c : c + 1].unsqueeze(1).broadcast_to((P, H, 1)),
            )

        # ---- q: natural LN on first QA rows -> qnT; qproj(0) ----
        def ln_q():
            nblk = (QA + P - 1) // P
            blocks = [(c, min(P, QA - c * P)) for c in range(nblk)]
            for i0 in range(0, len(blocks), 2):
                chunk = blocks[i0 : i0 + 2]
                cw = len(chunk)
                mv = stats.tile([P, 2, 2], F32, name="bnagg")
                for cc, (c, rows) in enumerate(chunk):
                    st = stats.tile([P, 6], F32, name="bnst")
                    nc.vector.bn_stats(out=st[:rows], in_=qf_s[:rows, c, :])
                    nc.vector.bn_aggr(out=mv[:rows, cc, :], in_=st[:rows])
                rstd = stats.tile([P, 2], F32, name="rstd")
                nc.scalar.activation(
                    out=rstd[:, :cw], in_=mv[:, 0:cw, 1],
                    func=AF.Abs_reciprocal_sqrt, bias=eps_t,
                )
                nm2 = stats.tile([P, 2], F32, name="nm2")
                nc.vector.tensor_tensor(
                    out=nm2[:, :cw], in0=mv[:, 0:cw, 0], in1=rstd[:, :cw], op=OP.mult
                )
                nc.vector.tensor_scalar_mul(nm2[:, :cw], nm2[:, :cw], -1.0)
                for cc, (c, rows) in enumerate(chunk):
                    xn = stats.tile([P, D], BF16, name="xnorm")
                    nc.vector.tensor_scalar(
                        out=xn[:rows],
                        in0=qf_s[:rows, c, :],
                        scalar1=nm2[:rows, cc : cc + 1],
                        scalar2=rstd[:rows, cc : cc + 1],
                        op0=OP.add,
                        op1=OP.mult,
                    )
                    pt = ptp.tile([P, ND, P], BF16, name="pt")
                    for b in range(ND):
                        nc.tensor.transpose(
                            pt[:, b, :rows],
                            xn[:rows, b * P : (b + 1) * P],
                            ident_bf[:rows, :rows],
                        )
                    nc.vector.tensor_copy(
                        qnT[:, :, c * P : c * P + rows], pt[:, :, :rows]
                    )

        ln_q()
        qproj_a(0, pp, "act")

        # ---- attention ----
        pst.release()
        pp.release()
        expp = tc.alloc_tile_pool(name="expp", bufs=3)
        psS = tc.alloc_tile_pool(name="psS", bufs=2, space="PSUM")
        pav = tc.alloc_tile_pool(name="pav", bufs=2, space="PSUM")
        pprj = tc.alloc_tile_pool(name="pprj", bufs=1, space="PSUM")

        DR = mybir.MatmulPerfMode.DoubleRow
        for nb in range(ND):
            proj_tasks = []
            if nb == 0:
                proj_tasks = [
                    lambda c=c: vproj_block(c, pprj) for c in range(2, NKC)
                ]
            if nb + 1 < ND:
                a1 = nb + 1
                proj_tasks += [
                    lambda a=a1, j0=j0: kproj_chunk(a, j0, pprj, "vec")
                    for j0 in range(0, KC, JC)
                ] + [lambda a=a1: qproj_a(a, pprj, "vec")]
            if nb == 0:
                vproj_block(0, pprj)
                vproj_block(1, pprj)
            av2 = [pav.tile([P, 512], F32, name="pav_t") for _ in range(2)]
            npairs = (NKC + 1) // 2
            for c0 in range(0, NKC, 2):
                cw = min(2, NKC - c0)
                # spread leftover projection/vproj work across the pairs
                rem_pairs = npairs - c0 // 2
                ntask = -(-len(proj_tasks) // rem_pairs)
                for _ in range(min(ntask, len(proj_tasks))):
                    proj_tasks.pop(0)()
                e8 = expp.tile([P, 2, 2, QA], FP8, name="expS")
                for cc in range(cw):
                    c = c0 + cc
                    ps2 = psS.tile([P, 2, 512], F32, name="pS2")
                    for i in range(2):
                        r0 = i * DH
                        nc.tensor.matmul(
                            ps2[:, i, :QA],
                            khT[nb][r0 : r0 + DH, c * P : (c + 1) * P],
                            qhT[nb][r0 : r0 + DH, :],
                            start=True,
                            stop=True,
                        )
                    nc.scalar.activation(
                        out=e8[:, cc, :, :], in_=ps2[:, :, :QA], func=AF.Exp,
                        scale=esck[:, c : c + 1], bias=eshift_t,
                    )
                for i in range(2):
                    if cw == 2:
                        nc.tensor.matmul(
                            av2[i][0 : DH + 1, :QA],
                            vh_aug[:, c0 : c0 + 2, 2 * nb + i, 0 : DH + 1],
                            e8[:, 0:2, i, :],
                            start=(c0 == 0),
                            stop=(c0 + 2 == NKC),
                            perf_mode=DR,
                        )
                    else:
                        nc.tensor.matmul(
                            av2[i][0 : DH + 1, :QA],
                            vh_aug[:, c0, 2 * nb + i, 0 : DH + 1],
                            e8[:, 0, i, :],
                            start=(c0 == 0),
                            stop=True,
                        )
            while proj_tasks:
                proj_tasks.pop(0)()
            # normalize: avT rows = [head even 0:64, head odd 64:128]
            for i in range(2):
                av = av2[i]
                avs = stats.tile([P, QA], BF16, name="avs")
                nc.vector.tensor_copy(avs[0 : DH + 1, :], av[0 : DH + 1, :QA])
                ftf = stats.tile([P, QA], F32, name="ftf")
                nc.vector.reciprocal(ftf[DH : DH + 1, :], avs[DH : DH + 1, :])
                ft = stats.tile([P, QA], BF16, name="ft")
                nc.vector.tensor_tensor(
                    out=ft[DH : DH + 1, :],
                    in0=ftf[DH : DH + 1, :],
                    in1=qm_bc[DH : DH + 1, :QA],
                    op=OP.mult,
                )
                fb = pprj.tile([P, 512], F32, name="prj_t")
                nc.tensor.matmul(
                    fb[0:DH, :QA],
                    ones_bf[DH : DH + 1, 0:DH],
                    ft[DH : DH + 1, :],
                    start=True,
                    stop=True,
                )
                fbs = stats.tile([P, QA], BF16, name="fbs")
                nc.vector.tensor_copy(fbs[0:DH, :], fb[0:DH, :QA])
                if i == 0:
                    nc.vector.tensor_tensor(
                        out=avT[0:DH, nb, :],
                        in0=avs[0:DH, :],
                        in1=fbs[0:DH, :],
                        op=OP.mult,
                    )
                else:
                    avtmp = stats.tile([P, QA], BF16, name="avtmp")
                    nc.vector.tensor_tensor(
                        out=avtmp[0:DH, :],
                        in0=avs[0:DH, :],
                        in1=fbs[0:DH, :],
                        op=OP.mult,
                    )
                    sh = pprj.tile([P, 512], F32, name="prj_t")
                    nc.tensor.matmul(
                        sh[DH : 2 * DH, :QA],
                        ident_bf[0:DH, 0:DH],
                        avtmp[0:DH, :],
                        start=True,
                        stop=True,
                    )
                    nc.vector.tensor_copy(avT[DH:P, nb, :], sh[DH : 2 * DH, :QA])

        # ---- output projection (natural): po[qblk] = avT.T @ Wo ----
        pprj.release()
        pav.release()
        psS.release()
        expp.release()
        pog = tc.alloc_tile_pool(name="pog", bufs=4, space="PSUM")
        NQB = (QA + P - 1) // P
        nc.gpsimd.memset(po[:, :, :], 0.0)
        for a in range(NQB):
            rows = min(P, QA - a * P)
            ps = pog.tile([P, 512], F32, name="pog_t")
            for b in range(ND):
                nc.tensor.matmul(
                    ps[:rows, :],
                    avT[:, b, a * P : a * P + rows],
                    w_s["wo"][:, b, :],
                    start=(b == 0),
                    stop=(b == ND - 1),
                )
            nc.vector.tensor_copy(po[:rows, a, :], ps[:rows, :])

        # ---- gate (natural): g = sigmoid(q@gwq + av_n@wop + gb) ----
        for a in range(ND):
            ps = pog.tile([P, 512], F32, name="pog_t")
            nc.tensor.matmul(
                ps, ones_bf[0:1, 0:P], gbn[0:1, :], start=True, stop=False
            )
            rows = min(max(QA - a * P, 0), P)
            for b in range(ND):
                nc.tensor.matmul(
                    ps,
                    qT[:, b, a * P : (a + 1) * P],
                    w_s["gwq"][:, b, :],
                    start=False,
                    stop=(b == ND - 1 and rows == 0),
                )
            if rows > 0:
                for b in range(ND):
                    nc.tensor.matmul(
                        ps[:rows, :],
                        avT[:, b, a * P : a * P + rows],
                        w_s["wop"][:, b, :],
                        start=False,
                        stop=(b == ND - 1),
                    )
            nc.scalar.activation(out=g_nat[:, a, :], in_=ps, func=AF.Sigmoid)

        # ---- combine: out = (q + po) + g*(q - po) ----
        for a in range(ND):
            s = stats.tile([P, D], BF16, name="fin_s")
            nc.vector.tensor_tensor(
                out=s, in0=qf_s[:, a, :], in1=po[:, a, :], op=OP.subtract
            )
            m = stats.tile([P, D], BF16, name="fin_m")
            nc.vector.tensor_tensor(out=m, in0=g_nat[:, a, :], in1=s, op=OP.mult)
            r = stats.tile([P, D], BF16, name="fin_r")
            nc.vector.tensor_tensor(
                out=r, in0=qf_s[:, a, :], in1=po[:, a, :], op=OP.add
            )
            nc.vector.tensor_tensor(out=outn[:, a, :], in0=m, in1=r, op=OP.add)

        dst = out_d[:, :].rearrange("(a p) d -> p a d", p=P)
        nc.sync.dma_start(out=dst, in_=outn)
        pog.release()
        ptp.release()


_CACHE: dict = {}


def _ceil(x, g):
    return -(-x // g) * g


def make_in_maps(inputs):
    q = np.asarray(inputs["query"], np.float32)
    k = np.asarray(inputs["key"], np.float32)
    v = np.asarray(inputs["value"], np.float32)
    wq = np.asarray(inputs["weight_q"], np.float32)
    wk = np.asarray(inputs["weight_k"], np.float32)
    wv = np.asarray(inputs["weight_v"], np.float32)
    wo = np.asarray(inputs["weight_o"], np.float32)
    gw = np.asarray(inputs["g_w"], np.float32)
    gb = np.asarray(inputs["g_b"], np.float32)
    qmask = np.asarray(inputs["query_mask"])
    kmask = np.asarray(inputs["key_mask"])
    gams = [np.asarray(inputs[n], np.float32) for n in ("q_gamma", "k_gamma", "v_gamma")]
    bets = [np.asarray(inputs[n], np.float32) for n in ("q_beta", "k_beta", "v_beta")]
    assert all(np.all(bt == 0.0) for bt in bets), "beta path not implemented"

    wqf = wq * gams[0][:, None]
    wkf = wk * gams[1][:, None]
    wvf = wv * gams[2][:, None]
    wopf = wo @ gw[D:, :]
    gwqf = gw[:D, :]

    per_batch = NCORES // B

    kidx = [np.where(kmask[b] != 0)[0] for b in range(B)]
    KC = _ceil(max(len(ix) + 1 for ix in kidx), P)
    NKC = KC // P
    kcs, vcs, kvals = [], [], []
    for b in range(B):
        ix = kidx[b]
        n = len(ix)
        kc = np.zeros((KC, D), np.float32)
        vc = np.zeros((KC, D), np.float32)
        kc[:n] = k[b][ix]
        vc[:n] = v[b][ix]
        kvc = np.zeros(KC, np.float32)
        kvc[: n + 1] = 1.0
        kcs.append(kc.astype(BF))
        vcs.append(vc.astype(BF))
        kvals.append(np.ascontiguousarray(kvc.reshape(NKC, P).T))

    rows_per_core = []
    na_per_core = []
    for b in range(B):
        un = np.where(qmask[b] != 0)[0]
        ma = np.where(qmask[b] == 0)[0]
        parts = [list(un[c::per_batch]) for c in range(per_batch)]
        mi = 0
        for c in range(per_batch):
            need = QS - len(parts[c])
            parts[c] = parts[c] + list(ma[mi : mi + need])
            mi += need
        assert mi == len(ma)
        for c in range(per_batch):
            rows_per_core.append(np.array(parts[c], np.int64))
            na_per_core.append(int((qmask[b][parts[c]] != 0).sum()))
    QA = max(_ceil(max(na_per_core), 16), 32)

    wmaps = {
        "wq": np.ascontiguousarray(wqf.astype(BF)),
        "wk": np.ascontiguousarray(wkf.astype(BF)),
        "wv": np.ascontiguousarray(wvf.astype(BF)),
        "wo": np.ascontiguousarray(wo.astype(BF)),
        "wop": np.ascontiguousarray(wopf.astype(BF)),
        "gwq": np.ascontiguousarray(gwqf.astype(BF)),
        "gbn": np.ascontiguousarray(gb.astype(BF)[None, :]),
        "w1kn": np.ascontiguousarray((-wkf.sum(axis=0)).astype(BF)[None, :]),
        "w1vn": np.ascontiguousarray((-wvf.sum(axis=0)).astype(BF)[None, :]),
    }

    in_maps = []
    for c in range(NCORES):
        b = c // per_batch
        rows = rows_per_core[c]
        m = dict(wmaps)
        m["qf"] = np.ascontiguousarray(q[b][rows]).astype(BF)
        m["kc"] = kcs[b]
        m["vc"] = vcs[b]
        m["kval"] = kvals[b]
        m["qm"] = qmask[b][rows].astype(np.float32)[None, :]
        in_maps.append(m)
    return in_maps, rows_per_core, (QA, KC)


def kernel(_return_res=False, _run_kwargs=None, **inputs):
    run_kwargs = _run_kwargs or {}
    in_maps, rows_per_core, key = make_in_maps(inputs)
    if key not in _CACHE:
        _CACHE[key] = _build(*key)
    nc = _CACHE[key]
    res = run_bass_kernel_spmd(nc, in_maps, list(range(NCORES)), **run_kwargs)
    out = np.empty((B, Q, D), np.float32)
    per_batch = NCORES // B
    for c in range(NCORES):
        b = c // per_batch
        out[b, rows_per_core[c]] = res.results[c]["out"].astype(np.float32)
    if _return_res:
        return out, res
    return out


# revision 41
# speedup vs baseline: 1.0782x; 1.0782x over previous
"""Trainium2 Bass kernel for BaseAttnPredictNet (pre-LN MHA with zero-attn
slot, gated output combination, residual).

v7 strategy (on top of host mask-compaction + fp8 PV + natural epilogue):
- k/v/qT are loaded PRE-TRANSPOSED from HBM via DMA-transpose; no natural
  staging, no PE transposes, no DVE layernorm passes for k/v.
- k/v LN is folded: row mean/meansq come from 1/512-valued ones matmuls on
  the PE, rstd = exp(-0.5*ln(var+eps)) on ACT, the mean correction enters
  the k/v projections as a rank-1 accumulation (-colsum(W) (x) mean), and
  the rstd scale rides the exp's per-partition scale (keys are partitions
  in the score block) resp. the vh copy scale.
- Attention: QK head-pair row-tiled bf16 matmuls -> shifted exp to fp8 ->
  DoubleRow fp8 PV with a valid-column producing the softmax denominator.
- Pair nb+1's k/q projections and the whole v projection pipeline are
  interleaved into the exp-paced attention stream.
- Epilogue in natural layout via activation-as-lhsT matmuls; gate bias via
  ones-row rank-1; all-bf16 combine.
"""

import numpy as np
import ml_dtypes

import concourse.bass as bass
import concourse.bacc as bacc
import concourse.mybir as mybir
import concourse.tile as tile
from concourse.bass_utils import run_bass_kernel_spmd
from concourse.masks import make_identity

B, Q, KLEN, D = 2, 2048, 2048, 512
H, DH = 8, 64
P = 128
QS = 512
ND = D // P
NCORES = 8
SCALE = 0.125
LN_EPS = 1e-5
ESHIFT = -4.5  # keeps exp within fp8 e4m3 range; softmax-invariant

F32 = mybir.dt.float32
BF16 = mybir.dt.bfloat16
AF = mybir.ActivationFunctionType
OP = mybir.AluOpType
BF = ml_dtypes.bfloat16
FP8 = mybir.dt.float8e4


def _build(QA: int, KC: int) -> bass.Bass:
    NKC = KC // P
    nc = bacc.Bacc("TRN2", target_bir_lowering=False, debug=False)

    din = {}
    for name, shape, dt in (
        ("qf", [QS, D], BF16),
        ("kc", [KC, D], BF16),
        ("vc", [KC, D], BF16),
        ("wq", [D, D], BF16),
        ("wk", [D, D], BF16),
        ("wv", [D, D], BF16),
        ("wo", [D, D], BF16),
        ("wop", [D, D], BF16),
        ("gwq", [D, D], BF16),
        ("gbn", [1, D], BF16),
        ("w1kn", [1, D], BF16),
        ("w1vn", [1, D], BF16),
        ("kval", [P, NKC], F32),
        ("qm", [1, QS], F32),
    ):
        din[name] = nc.dram_tensor(name, shape, dt, kind="ExternalInput")
    out_d = nc.dram_tensor("out", [QS, D], BF16, kind="ExternalOutput")

    with tile.TileContext(nc) as tc:
        _body(nc, tc, din, out_d, QA, KC)
    nc.compile()
    return nc


def _body(nc, tc, din, out_d, QA, KC):
    NKC = KC // P
    JC = KC // 3
    from contextlib import ExitStack

    ctx = ExitStack()
    with ctx:
        persist = ctx.enter_context(tc.tile_pool(name="persist", bufs=1))
        stats = ctx.enter_context(tc.tile_pool(name="stats", bufs=6))

        ident_bf = persist.tile([P, P], BF16)
        make_identity(nc, ident_bf)
        ident_f32 = persist.tile([P, P], F32)
        make_identity(nc, ident_f32)
        ones_bf = persist.tile([P, P], BF16)
        nc.vector.memset(ones_bf, 1.0)
        oneN = persist.tile([P, 1], BF16)
        nc.vector.memset(oneN, 1.0 / D)
        eps_t = persist.tile([P, 1], F32)
        nc.vector.memset(eps_t, LN_EPS)
        eshift_t = persist.tile([P, 1], F32)
        nc.vector.memset(eshift_t, ESHIFT)

        kval = persist.tile([P, NKC], F32)
        gbn = persist.tile([1, D], BF16)
        w1kn = persist.tile([1, D], BF16)
        w1vn = persist.tile([1, D], BF16)
        qm_bc = persist.tile([P, QS], F32)

        # ---- persistent activations ----
        kT = persist.tile([P, ND, KC], BF16)   # raw k, feature-major
        vT = persist.tile([P, ND, KC], BF16)
        vh_aug = persist.tile([P, NKC, H, DH + 2], FP8)
        qnT = persist.tile([P, ND, QA], BF16)
        qT = persist.tile([P, ND, QS], BF16)
        qhT = [persist.tile([P, QA], BF16, name=f"qhT{a}") for a in range(ND)]
        khT = [persist.tile([P, KC], BF16, name=f"khT{a}") for a in range(ND)]
        avT = persist.tile([P, ND, QA], BF16)
        po = persist.tile([P, ND, D], BF16)
        g_nat = persist.tile([P, ND, D], BF16)
        outn = persist.tile([P, ND, D], BF16)
        qf_s = persist.tile([P, ND, D], BF16)
        esck = persist.tile([P, NKC], F32)   # exp scale: SCALE*rstd_k
        vscv = persist.tile([P, NKC], F32)   # vh scale: rstd_v*kval

        # ---- DMAs split across the two HWDGE engines (sync + ACT) and
        # SWDGE (gpsimd) so trigger overhead doesn't serialize; critical
        # weights first ----
        w_s = {}
        for wname in ("wk", "wq", "wv", "wo", "wop", "gwq"):
            w_s[wname] = persist.tile([P, ND, D], BF16, name=f"{wname}_s")

        def wdma(eng, wname):
            eng.dma_start(
                out=w_s[wname],
                in_=din[wname][:, :].rearrange("(b p) d -> p b d", p=P),
            )

        wdma(nc.sync, "wk")
        wdma(nc.scalar, "wv")
        for b in range(ND):
            nc.sync.dma_start_transpose(
                out=kT[:, b, :], in_=din["kc"][:, b * P : (b + 1) * P]
            )
            nc.scalar.dma_start_transpose(
                out=vT[:, b, :], in_=din["vc"][:, b * P : (b + 1) * P]
            )
        nc.sync.dma_start(
            out=qf_s, in_=din["qf"][:, :].rearrange("(a p) d -> p a d", p=P)
        )
        wdma(nc.sync, "wq")
        nc.gpsimd.dma_start(out=kval, in_=din["kval"][:, :])
        nc.gpsimd.dma_start(out=w1kn, in_=din["w1kn"][:, :])
        nc.gpsimd.dma_start(out=w1vn, in_=din["w1vn"][:, :])
        nc.gpsimd.dma_start(out=gbn, in_=din["gbn"][:, :])
        _qm_ap = din["qm"][:, :]
        nc.gpsimd.dma_start(
            out=qm_bc,
            in_=bass.AP(tensor=_qm_ap.tensor, offset=_qm_ap.offset, ap=[[0, P], [1, QS]]),
        )
        for b in range(ND):
            nc.scalar.dma_start_transpose(
                out=qT[:, b, :], in_=din["qf"][:, b * P : (b + 1) * P]
            )
        wdma(nc.scalar, "wo")
        wdma(nc.scalar, "wop")
        wdma(nc.scalar, "gwq")

        ptp = tc.alloc_tile_pool(name="ptp", bufs=1, space="PSUM")
        pp = tc.alloc_tile_pool(name="pp", bufs=2, space="PSUM")
        pst = tc.alloc_tile_pool(name="pst", bufs=2, space="PSUM")

        def fold_stats(xT, dst_pn):
            """Row LN stats from feature-major xT: means (returned as [1,KC]
            SBUF bf16 for rank-1 use) and rstd transposed into dst_pn [P,NKC]
            via a DRAM round-trip."""
            m_row = stats.tile([1, KC], F32, name="m_row", bufs=2)
            ms_row = stats.tile([1, KC], F32, name="ms_row", bufs=2)
            sq = stats.tile([P, ND, JC], BF16, name="sq", bufs=2)
            for j0 in range(0, KC, JC):
                psm = pst.tile([1, 512], F32, name="pst_t")
                for b in range(ND):
                    nc.tensor.matmul(
                        psm[:, :JC],
                        oneN[:, :],
                        xT[:, b, j0 : j0 + JC],
                        start=(b == 0),
                        stop=(b == ND - 1),
                    )
                nc.vector.tensor_copy(m_row[:, j0 : j0 + JC], psm[:, :JC])
                for b in range(ND):
                    nc.scalar.activation(
                        out=sq[:, b, :], in_=xT[:, b, j0 : j0 + JC], func=AF.Square
                    )
                psq = pst.tile([1, 512], F32, name="pst_t")
                for b in range(ND):
                    nc.tensor.matmul(
                        psq[:, :JC],
                        oneN[:, :],
                        sq[:, b, :],
                        start=(b == 0),
                        stop=(b == ND - 1),
                    )
                nc.vector.tensor_copy(ms_row[:, j0 : j0 + JC], psq[:, :JC])
            m_bf = stats.tile([1, KC], BF16, name="m_bf", bufs=2)
            nc.vector.tensor_copy(m_bf, m_row)
            var = stats.tile([1, KC], F32, name="var", bufs=2)
            nc.vector.tensor_tensor(out=var, in0=m_row, in1=m_row, op=OP.mult)
            nc.vector.tensor_tensor(out=var, in0=ms_row, in1=var, op=OP.subtract)
            rst_row = stats.tile([1, KC], F32, name="rst_row", bufs=2)
            nc.scalar.activation(
                out=rst_row, in_=var, func=AF.Abs_reciprocal_sqrt, bias=eps_t[0:1, :]
            )
            # layout flip [1,KC] -> [P,NKC]: tiny PE transposes, one per block
            prt = pst.tile([P, NKC], F32, name="pst_f")
            for c in range(NKC):
                nc.tensor.transpose(
                    prt[:, c : c + 1],
                    rst_row[0:1, c * P : (c + 1) * P],
                    ident_f32[0:1, 0:1],
                )
            nc.vector.tensor_copy(dst_pn, prt)
            return m_bf

        # ---- k: stats + projection (a=0 now; a>0 interleaved later) ----
        km_bf = fold_stats(kT, esck)
        nc.vector.tensor_scalar_mul(esck, esck, SCALE)

        def kproj_chunk(a, j0, pool, copy_eng):
            ps = pool.tile([P, 512], F32, name="prj_t")
            for b in range(ND):
                nc.tensor.matmul(
                    ps[:, :JC],
                    w_s["wk"][:, b, a * P : (a + 1) * P],
                    kT[:, b, j0 : j0 + JC],
                    start=(b == 0),
                    stop=False,
                )
            nc.tensor.matmul(
                ps[:, :JC],
                w1kn[0:1, a * P : (a + 1) * P],
                km_bf[0:1, j0 : j0 + JC],
                start=False,
                stop=True,
            )
            if copy_eng == "act":
                nc.scalar.copy(khT[a][:, j0 : j0 + JC], ps[:, :JC])
            else:
                nc.vector.tensor_copy(khT[a][:, j0 : j0 + JC], ps[:, :JC])

        def qproj_a(a, pool, copy_eng):
            ps = pool.tile([P, 512], F32, name="prj_t")
            for b in range(ND):
                nc.tensor.matmul(
                    ps[:, :QA],
                    w_s["wq"][:, b, a * P : (a + 1) * P],
                    qnT[:, b, :],
                    start=(b == 0),
                    stop=(b == ND - 1),
                )
            if copy_eng == "act":
                nc.scalar.copy(qhT[a], ps[:, :QA])
            else:
                nc.vector.tensor_copy(qhT[a], ps[:, :QA])

        for j0 in range(0, KC, JC):
            kproj_chunk(0, j0, pp, "act")

        # ---- v stats (vproj itself is interleaved into attention nb=0) ----
        vm_bf = fold_stats(vT, vscv)
        nc.vector.tensor_tensor(out=vscv, in0=vscv, in1=kval, op=OP.mult)

        def vproj_block(c, pool):
            ps = pool.tile([P, 512], F32, name="prj_t")
            for b in range(ND):
                nc.tensor.matmul(
                    ps,
                    vT[:, b, c * P : (c + 1) * P],
                    w_s["wv"][:, b, :],
                    start=(b == 0),
                    stop=False,
                )
            nc.tensor.matmul(
                ps,
                vm_bf[0:1, c * P : (c + 1) * P],
                w1vn[0:1, :],
                start=False,
                stop=True,
            )
            pp3 = ps.rearrange("p (h e) -> p h e", h=H)
            nc.vector.tensor_scalar(
                out=vh_aug[:, c, :, 0:DH],
                in0=pp3,
                scalar1=vscv[:, c : c + 1],
                scalar2=None,
                op0=OP.mult,
            )
            nc.vector.tensor_copy(
                vh_aug[:, c, :, DH : DH + 1],
                kval[:, c : c + 1].unsqueeze(1).broadcast_to((P, H, 1)),
            )

        # ---- q: natural LN on first QA rows -> qnT; qproj(0) ----
        def ln_q():
            nblk = (QA + P - 1) // P
            blocks = [(c, min(P, QA - c * P)) for c in range(nblk)]
            for i0 in range(0, len(blocks), 2):
                chunk = blocks[i0 : i0 + 2]
                cw = len(chunk)
                mv = stats.tile([P, 2, 2], F32, name="bnagg")
                for cc, (c, rows) in enumerate(chunk):
                    st = stats.tile([P, 6], F32, name="bnst")
                    nc.vector.bn_stats(out=st[:rows], in_=qf_s[:rows, c, :])
                    nc.vector.bn_aggr(out=mv[:rows, cc, :], in_=st[:rows])
                rstd = stats.tile([P, 2], F32, name="rstd")
                nc.scalar.activation(
                    out=rstd[:, :cw], in_=mv[:, 0:cw, 1],
                    func=AF.Abs_reciprocal_sqrt, bias=eps_t,
                )
                nm2 = stats.tile([P, 2], F32, name="nm2")
                nc.vector.tensor_tensor(
                    out=nm2[:, :cw], in0=mv[:, 0:cw, 0], in1=rstd[:, :cw], op=OP.mult
                )
                nc.vector.tensor_scalar_mul(nm2[:, :cw], nm2[:, :cw], -1.0)
                for cc, (c, rows) in enumerate(chunk):
                    xn = stats.tile([P, D], BF16, name="xnorm")
                    nc.vector.tensor_scalar(
                        out=xn[:rows],
                        in0=qf_s[:rows, c, :],
                        scalar1=nm2[:rows, cc : cc + 1],
                        scalar2=rstd[:rows, cc : cc + 1],
                        op0=OP.add,
                        op1=OP.mult,
                    )
                    pt = ptp.tile([P, ND, P], BF16, name="pt")
                    for b in range(ND):
                        nc.tensor.transpose(
                            pt[:, b, :rows],
                            xn[:rows, b * P : (b + 1) * P],
                            ident_bf[:rows, :rows],
                        )
                    nc.vector.tensor_copy(
                        qnT[:, :, c * P : c * P + rows], pt[:, :, :rows]
                    )

        ln_q()
        qproj_a(0, pp, "act")

        # ---- attention ----
        pst.release()
        pp.release()
        expp = tc.alloc_tile_pool(name="expp", bufs=3)
        psS = tc.alloc_tile_pool(name="psS", bufs=2, space="PSUM")
        pav = tc.alloc_tile_pool(name="pav", bufs=2, space="PSUM")
        pprj = tc.alloc_tile_pool(name="pprj", bufs=1, space="PSUM")

        DR = mybir.MatmulPerfMode.DoubleRow
        for nb in range(ND):
            proj_tasks = []
            if nb == 0:
                proj_tasks = [
                    lambda c=c: vproj_block(c, pprj) for c in range(2, NKC)
                ]
            if nb + 1 < ND:
                a1 = nb + 1
                proj_tasks += [
                    lambda a=a1, j0=j0: kproj_chunk(a, j0, pprj, "vec")
                    for j0 in range(0, KC, JC)
                ] + [lambda a=a1: qproj_a(a, pprj, "vec")]
            if nb == 0:
                vproj_block(0, pprj)
                vproj_block(1, pprj)
            av2 = [pav.tile([P, 512], F32, name="pav_t") for _ in range(2)]
            npairs = (NKC + 1) // 2
            for c0 in range(0, NKC, 2):
                cw = min(2, NKC - c0)
                # spread leftover projection/vproj work across the pairs
                rem_pairs = npairs - c0 // 2
                ntask = -(-len(proj_tasks) // rem_pairs)
                for _ in range(min(ntask, len(proj_tasks))):
                    proj_tasks.pop(0)()
                e8 = expp.tile([P, 2, 2, QA], FP8, name="expS")
                for cc in range(cw):
                    c = c0 + cc
                    ps2 = psS.tile([P, 2, 512], F32, name="pS2")
                    for i in range(2):
                        r0 = i * DH
                        nc.tensor.matmul(
                            ps2[:, i, :QA],
                            khT[nb][r0 : r0 + DH, c * P : (c + 1) * P],
                            qhT[nb][r0 : r0 + DH, :],
                            start=True,
                            stop=True,
                        )
                    nc.scalar.activation(
                        out=e8[:, cc, :, :], in_=ps2[:, :, :QA], func=AF.Exp,
                        scale=esck[:, c : c + 1], bias=eshift_t,
                    )
                for i in range(2):
                    if cw == 2:
                        nc.tensor.matmul(
                            av2[i][0 : DH + 1, :QA],
                            vh_aug[:, c0 : c0 + 2, 2 * nb + i, 0 : DH + 1],
                            e8[:, 0:2, i, :],
                            start=(c0 == 0),
                            stop=(c0 + 2 == NKC),
                            perf_mode=DR,
                        )
                    else:
                        nc.tensor.matmul(
                            av2[i][0 : DH + 1, :QA],
                            vh_aug[:, c0, 2 * nb + i, 0 : DH + 1],
                            e8[:, 0, i, :],
                            start=(c0 == 0),
                            stop=True,
                        )
            while proj_tasks:
                proj_tasks.pop(0)()
            # normalize: avT rows = [head even 0:64, head odd 64:128]
            for i in range(2):
                av = av2[i]
                avs = stats.tile([P, QA], BF16, name="avs")
                nc.vector.tensor_copy(avs[0 : DH + 1, :], av[0 : DH + 1, :QA])
                ftf = stats.tile([P, QA], F32, name="ftf")
                nc.vector.reciprocal(ftf[DH : DH + 1, :], avs[DH : DH + 1, :])
                ft = stats.tile([P, QA], BF16, name="ft")
                nc.vector.tensor_tensor(
                    out=ft[DH : DH + 1, :],
                    in0=ftf[DH : DH + 1, :],
                    in1=qm_bc[DH : DH + 1, :QA],
                    op=OP.mult,
                )
                fb = pprj.tile([P, 512], F32, name="prj_t")
                nc.tensor.matmul(
                    fb[0:DH, :QA],
                    ones_bf[DH : DH + 1, 0:DH],
                    ft[DH : DH + 1, :],
                    start=True,
                    stop=True,
                )
                fbs = stats.tile([P, QA], BF16, name="fbs")
                nc.vector.tensor_copy(fbs[0:DH, :], fb[0:DH, :QA])
                if i == 0:
                    nc.vector.tensor_tensor(
                        out=avT[0:DH, nb, :],
                        in0=avs[0:DH, :],
                        in1=fbs[0:DH, :],
                        op=OP.mult,
                    )
                else:
                    avtmp = stats.tile([P, QA], BF16, name="avtmp")
                    nc.vector.tensor_tensor(
                        out=avtmp[0:DH, :],
                        in0=avs[0:DH, :],
                        in1=fbs[0:DH, :],
                        op=OP.mult,
                    )
                    sh = pprj.tile([P, 512], F32, name="prj_t")
                    nc.tensor.matmul(
                        sh[DH : 2 * DH, :QA],
                        ident_bf[0:DH, 0:DH],
                        avtmp[0:DH, :],
                        start=True,
                        stop=True,
                    )
                    nc.vector.tensor_copy(avT[DH:P, nb, :], sh[DH : 2 * DH, :QA])

        # ---- output projection (natural): po[qblk] = avT.T @ Wo ----
        pprj.release()
        pav.release()
        psS.release()
        expp.release()
        pog = tc.alloc_tile_pool(name="pog", bufs=4, space="PSUM")
        NQB = (QA + P - 1) // P
        nc.gpsimd.memset(po[:, :, :], 0.0)
        for a in range(NQB):
            rows = min(P, QA - a * P)
            ps = pog.tile([P, 512], F32, name="pog_t")
            for b in range(ND):
                nc.tensor.matmul(
                    ps[:rows, :],
                    avT[:, b, a * P : a * P + rows],
                    w_s["wo"][:, b, :],
                    start=(b == 0),
                    stop=(b == ND - 1),
                )
            nc.vector.tensor_copy(po[:rows, a, :], ps[:rows, :])

        # ---- gate (natural): g = sigmoid(q@gwq + av_n@wop + gb) ----
        for a in range(ND):
            ps = pog.tile([P, 512], F32, name="pog_t")
            nc.tensor.matmul(
                ps, ones_bf[0:1, 0:P], gbn[0:1, :], start=True, stop=False
            )
            rows = min(max(QA - a * P, 0), P)
            for b in range(ND):
                nc.tensor.matmul(
                    ps,
                    qT[:, b, a * P : (a + 1) * P],
                    w_s["gwq"][:, b, :],
                    start=False,
                    stop=(b == ND - 1 and rows == 0),
                )
            if rows > 0:
                for b in range(ND):
                    nc.tensor.matmul(
                        ps[:rows, :],
                        avT[:, b, a * P : a * P + rows],
                        w_s["wop"][:, b, :],
                        start=False,
                        stop=(b == ND - 1),
                    )
            nc.scalar.activation(out=g_nat[:, a, :], in_=ps, func=AF.Sigmoid)

        # ---- combine: out = (q + po) + g*(q - po) ----
        for a in range(ND):
            s = stats.tile([P, D], BF16, name="fin_s")
            nc.vector.tensor_tensor(
                out=s, in0=qf_s[:, a, :], in1=po[:, a, :], op=OP.subtract
            )
            m = stats.tile([P, D], BF16, name="fin_m")
            nc.vector.tensor_tensor(out=m, in0=g_nat[:, a, :], in1=s, op=OP.mult)
            r = stats.tile([P, D], BF16, name="fin_r")
            nc.vector.tensor_tensor(
                out=r, in0=qf_s[:, a, :], in1=po[:, a, :], op=OP.add
            )
            nc.vector.tensor_tensor(out=outn[:, a, :], in0=m, in1=r, op=OP.add)

        dst = out_d[:, :].rearrange("(a p) d -> p a d", p=P)
        nc.sync.dma_start(out=dst, in_=outn)
        pog.release()
        ptp.release()


_CACHE: dict = {}


def _ceil(x, g):
    return -(-x // g) * g


def make_in_maps(inputs):
    q = np.asarray(inputs["query"], np.float32)
    k = np.asarray(inputs["key"], np.float32)
    v = np.asarray(inputs["value"], np.float32)
    wq = np.asarray(inputs["weight_q"], np.float32)
    wk = np.asarray(inputs["weight_k"], np.float32)
    wv = np.asarray(inputs["weight_v"], np.float32)
    wo = np.asarray(inputs["weight_o"], np.float32)
    gw = np.asarray(inputs["g_w"], np.float32)
    gb = np.asarray(inputs["g_b"], np.float32)
    qmask = np.asarray(inputs["query_mask"])
    kmask = np.asarray(inputs["key_mask"])
    gams = [np.asarray(inputs[n], np.float32) for n in ("q_gamma", "k_gamma", "v_gamma")]
    bets = [np.asarray(inputs[n], np.float32) for n in ("q_beta", "k_beta", "v_beta")]
    assert all(np.all(bt == 0.0) for bt in bets), "beta path not implemented"

    wqf = wq * gams[0][:, None]
    wkf = wk * gams[1][:, None]
    wvf = wv * gams[2][:, None]
    wopf = wo @ gw[D:, :]
    gwqf = gw[:D, :]

    per_batch = NCORES // B

    kidx = [np.where(kmask[b] != 0)[0] for b in range(B)]
    KC = _ceil(max(len(ix) + 1 for ix in kidx), P)
    NKC = KC // P
    kcs, vcs, kvals = [], [], []
    for b in range(B):
        ix = kidx[b]
        n = len(ix)
        kc = np.zeros((KC, D), np.float32)
        vc = np.zeros((KC, D), np.float32)
        kc[:n] = k[b][ix]
        vc[:n] = v[b][ix]
        kvc = np.zeros(KC, np.float32)
        kvc[: n + 1] = 1.0
        kcs.append(kc.astype(BF))
        vcs.append(vc.astype(BF))
        kvals.append(np.ascontiguousarray(kvc.reshape(NKC, P).T))

    rows_per_core = []
    na_per_core = []
    for b in range(B):
        un = np.where(qmask[b] != 0)[0]
        ma = np.where(qmask[b] == 0)[0]
        parts = [list(un[c::per_batch]) for c in range(per_batch)]
        mi = 0
        for c in range(per_batch):
            need = QS - len(parts[c])
            parts[c] = parts[c] + list(ma[mi : mi + need])
            mi += need
        assert mi == len(ma)
        for c in range(per_batch):
            rows_per_core.append(np.array(parts[c], np.int64))
            na_per_core.append(int((qmask[b][parts[c]] != 0).sum()))
    QA = max(_ceil(max(na_per_core), 16), 32)

    wmaps = {
        "wq": np.ascontiguousarray(wqf.astype(BF)),
        "wk": np.ascontiguousarray(wkf.astype(BF)),
        "wv": np.ascontiguousarray(wvf.astype(BF)),
        "wo": np.ascontiguousarray(wo.astype(BF)),
        "wop": np.ascontiguousarray(wopf.astype(BF)),
        "gwq": np.ascontiguousarray(gwqf.astype(BF)),
        "gbn": np.ascontiguousarray(gb.astype(BF)[None, :]),
        "w1kn": np.ascontiguousarray((-wkf.sum(axis=0)).astype(BF)[None, :]),
        "w1vn": np.ascontiguousarray((-wvf.sum(axis=0)).astype(BF)[None, :]),
    }

    in_maps = []
    for c in range(NCORES):
        b = c // per_batch
        rows = rows_per_core[c]
        m = dict(wmaps)
        m["qf"] = np.ascontiguousarray(q[b][rows]).astype(BF)
        m["kc"] = kcs[b]
        m["vc"] = vcs[b]
        m["kval"] = kvals[b]
        m["qm"] = qmask[b][rows].astype(np.float32)[None, :]
        in_maps.append(m)
    return in_maps, rows_per_core, (QA, KC)


def kernel(_return_res=False, _run_kwargs=None, **inputs):
    run_kwargs = _run_kwargs or {}
    in_maps, rows_per_core, key = make_in_maps(inputs)
    if key not in _CACHE:
        _CACHE[key] = _build(*key)
    nc = _CACHE[key]
    res = run_bass_kernel_spmd(nc, in_maps, list(range(NCORES)), **run_kwargs)
    out = np.empty((B, Q, D), np.float32)
    per_batch = NCORES // B
    for c in range(NCORES):
        b = c // per_batch
        out[b, rows_per_core[c]] = res.results[c]["out"].astype(np.float32)
    if _return_res:
        return out, res
    return out
